# revision 1
# baseline (speedup 1.0000x reference)
"""Trainium2 Bass kernel for nn_NoFoDifformer_FourierKAN (8-core SPMD).

Sharding: u and nodes row-wise across 8 cores (1250 rows each). The [d,d]
K^T V Gram matrices and the chunked u^T h partial sums are all-reduced;
small weights are replicated; per-core row-shard outputs are concatenated
on the host.

The spectral sandwich u @ diag(new_e) @ (u^T @ h) reads u from HBM ONCE per
core: pass-1 (u^T h partials) consumes row-major fp32 tiles with float32r
matmuls; the same SBUF tiles are cast to bf16 (GPSIMD) and PE-transposed
into SBUF-resident uT chunks that pass-2 consumes after the chunked
all-reduce of the pass-1 partials. LayerNorm affine params are folded into
the downstream projection weights so on-chip LN is normalize-only.
"""

import math
from contextlib import ExitStack

import numpy as np

N_FULL = 10000
NF_FULL = 512
D = 128
CORES_FULL = 8
CHUNK_FULL = 1280
LAMBDA_INIT = 0.2


def _ceil_div(a, b):
    return (a + b - 1) // b


def _splits(total, step):
    return [(o, min(step, total - o)) for o in range(0, total, step)]


def build_kernel(N=N_FULL, NF=NF_FULL, CORES=CORES_FULL, CHUNK=CHUNK_FULL,
                 sim_gelu=False):
    import concourse.bacc as bacc
    import concourse.tile as tile
    from concourse import mybir
    from concourse.masks import make_identity

    dt = mybir.dt
    f32 = dt.float32
    f32r = dt.float32r
    bf16 = dt.bfloat16
    AF = mybir.ActivationFunctionType
    ALU = mybir.AluOpType
    AX = mybir.AxisListType

    NLOC = N // CORES
    assert NLOC * CORES == N
    ROWS = _splits(NLOC, 128)          # per-core row tiles (i)
    NR = len(ROWS)
    KX = NF // 128                     # x feature k-tiles
    assert KX * 128 == NF
    CHUNKS = _splits(N, CHUNK)         # j chunks
    TG_FULL = N // 128                 # full 128-wide j-subtiles globally
    TG_REM = N - TG_FULL * 128
    NSUB = _ceil_div(N, 128)           # total j subtiles
    NSUB_C = _ceil_div(CHUNK, 128)     # max j subtiles per chunk
    IBLK = _splits(NLOC, 512)          # pass-2 output i blocks
    RG = _splits(NLOC, 512)            # phase-A row groups
    assert TG_FULL <= 128
    rg = [list(range(CORES))]
    shared_space = "Shared" if CORES > 4 else "Local"

    nc = bacc.Bacc("TRN2", target_bir_lowering=False, debug=False,
                   num_devices=CORES)

    # ---------------- DRAM I/O ----------------
    def din(name, shape):
        return nc.dram_tensor(name, list(shape), f32, kind="ExternalInput")

    t_x = din("x", (NLOC, NF))
    t_u = nc.dram_tensor("u", [NLOC, N], f32r, kind="ExternalInput")
    t_e = din("e", (N,))
    t_few1 = din("fe_w1", (NF, D)); t_feb1 = din("fe_b1", (D,))
    t_few2 = din("fe_w2", (D, D)); t_feb2 = din("fe_b2", (D,))
    t_kana = din("kan_a", (10,)); t_kanb = din("kan_b", (10,))
    t_kanbias = din("kan_bias", (1,)); t_alpha = din("alpha_w", (1, 1))
    t_mg = din("mha_ln_g", (D,)); t_mb = din("mha_ln_b", (D,))
    t_fg = din("ffn_ln_g", (D,)); t_fb = din("ffn_ln_b", (D,))
    t_q1w = din("q1_w", (D, D)); t_q1b = din("q1_b", (D,))
    t_k1w = din("k1_w", (D, D)); t_k1b = din("k1_b", (D,))
    t_q2w = din("q2_w", (D, D)); t_q2b = din("q2_b", (D,))
    t_k2w = din("k2_w", (D, D)); t_k2b = din("k2_b", (D,))
    t_vw = din("v_w", (D, D)); t_vb = din("v_b", (D,))
    t_ag = din("attn_ln_g", (D,)); t_ab = din("attn_ln_b", (D,))
    t_ow = din("out_w", (D, D)); t_ob = din("out_b", (D,))
    t_lq1 = din("lq1", (D,)); t_lk1 = din("lk1", (D,))
    t_lq2 = din("lq2", (D,)); t_lk2 = din("lk2", (D,))
    t_f1w = din("ffn1_w", (D, D)); t_f1b = din("ffn1_b", (D,))
    t_f2w = din("ffn2_w", (D, D)); t_f2b = din("ffn2_b", (D,))
    t_out = nc.dram_tensor("out", [NLOC, D], f32, kind="ExternalOutput")

    with tile.TileContext(nc) as tc, ExitStack() as ctx:
        wpool = ctx.enter_context(tc.tile_pool(name="wpool", bufs=1))
        rowtmp = ctx.enter_context(tc.tile_pool(name="rowtmp", bufs=3))
        ustream = ctx.enter_context(tc.tile_pool(name="ustream", bufs=14))
        uTt = ctx.enter_context(tc.tile_pool(name="uTt", bufs=12))
        zp = ctx.enter_context(tc.tile_pool(name="zp", bufs=2))
        stg = ctx.enter_context(tc.tile_pool(name="stg", bufs=2))
        dram = ctx.enter_context(tc.tile_pool(name="dram", bufs=1, space="DRAM"))
        ps_p1 = ctx.enter_context(tc.tile_pool(name="ps_p1", bufs=3, space="PSUM"))
        ps_p2 = ctx.enter_context(tc.tile_pool(name="ps_p2", bufs=3, space="PSUM"))
        ps_mm = ctx.enter_context(tc.tile_pool(name="ps_mm", bufs=2, space="PSUM"))

        def p1_tile(w):
            return ps_p1.tile([128, 512], f32, tag="p1",
                              name=f"p1_{nc.next_id()}")[:, :w]

        def p2_tile(w):
            return ps_p2.tile([128, 512], f32, tag="p2",
                              name=f"p2_{nc.next_id()}")[:, :w]

        def mm_tile(p, w):
            return ps_mm.tile([128, 512], f32, tag="mmp",
                              name=f"mm_{nc.next_id()}")[:p, :w]

        def wtile(shape, dtype, name):
            return wpool.tile(shape, dtype, tag=name, name=name)

        def rtile(shape, dtype, tag):
            return rowtmp.tile(shape, dtype, tag=tag,
                               name=f"{tag}_{nc.next_id()}")

        def T(out_psum, in_sbuf, identity):
            nc.tensor.matmul(out_psum, in_sbuf, identity, is_transpose=True)

        # ================= constants & weights =================
        ident = wtile([128, 128], f32, "ident")
        make_identity(nc, ident[:])
        identb = wtile([128, 128], bf16, "identb")
        make_identity(nc, identb[:])

        ones_row = wtile([1, 128], f32, "ones_row")
        nc.vector.memset(ones_row[:], 1.0)
        eps_col = wtile([128, 1], f32, "eps_col")
        nc.vector.memset(eps_col[:], 1e-5)
        c08_col = wtile([128, 1], f32, "c08_col")
        nc.vector.memset(c08_col[:], 1.0 - LAMBDA_INIT)
        one_col = wtile([128, 1], f32, "one_col")
        nc.vector.memset(one_col[:], 1.0)
        laminit_c = wtile([1, 1], f32, "laminit_c")
        nc.vector.memset(laminit_c[:], LAMBDA_INIT)

        def ldw(name, dram_t, shape, rearr=None, **kw):
            t = wtile(shape, f32, name)
            src = dram_t[:] if rearr is None else dram_t[:].rearrange(rearr, **kw)
            nc.scalar.dma_start(out=t[:], in_=src)
            return t

        few1 = ldw("few1", t_few1, [128, KX, D], "(t p) d -> p t d", p=128)
        few2 = ldw("few2", t_few2, [128, D])
        q1w = ldw("q1w", t_q1w, [128, D])
        k1w = ldw("k1w", t_k1w, [128, D])
        q2w = ldw("q2w", t_q2w, [128, D])
        k2w = ldw("k2w", t_k2w, [128, D])
        vw = ldw("vw", t_vw, [128, D])
        ow = ldw("ow", t_ow, [128, D])
        f1w = ldw("f1w", t_f1w, [128, D])
        f2w = ldw("f2w", t_f2w, [128, D])

        def ldcol(name, dram_t):
            t = wtile([128, 1], f32, name)
            nc.scalar.dma_start(out=t[:],
                                in_=dram_t[:].rearrange("(p x) -> p x", x=1))
            return t

        feb1_c = ldcol("feb1_c", t_feb1)
        mg_c = ldcol("mg_c", t_mg); mb_c = ldcol("mb_c", t_mb)
        fg_c = ldcol("fg_c", t_fg); fb_c = ldcol("fb_c", t_fb)
        ag_c = ldcol("ag_c", t_ag); ab_c = ldcol("ab_c", t_ab)
        q1b_c = ldcol("q1b_c", t_q1b); q2b_c = ldcol("q2b_c", t_q2b)

        def ldrow(name, dram_t, w=128):
            t = wtile([1, w], f32, name)
            nc.scalar.dma_start(out=t[:],
                                in_=dram_t[:].rearrange("(x p) -> x p", x=1))
            return t

        k1b_r = ldrow("k1b_r", t_k1b); k2b_r = ldrow("k2b_r", t_k2b)
        vb_r = ldrow("vb_r", t_vb); ob_r = ldrow("ob_r", t_ob)
        f1b_r = ldrow("f1b_r", t_f1b)
        lq1_r = ldrow("lq1_r", t_lq1); lk1_r = ldrow("lk1_r", t_lk1)
        lq2_r = ldrow("lq2_r", t_lq2); lk2_r = ldrow("lk2_r", t_lk2)
        kana_r = ldrow("kana_r", t_kana, 10)
        kanb_r = ldrow("kanb_r", t_kanb, 10)
        kbias_r = ldrow("kbias_r", t_kanbias, 1)
        alpha_r = wtile([1, 1], f32, "alpha_r")
        nc.scalar.dma_start(out=alpha_r[:], in_=t_alpha[:])

        def ldbcast(name, dram_t):
            t = wtile([128, D], f32, name)
            nc.scalar.dma_start(out=t[:], in_=dram_t[:].partition_broadcast(128))
            return t

        feb2_B = ldbcast("feb2_B", t_feb2)
        f2b_B = ldbcast("f2b_B", t_f2b)

        # ---------- scalars: lambda ----------
        srow = wtile([1, 8], f32, "srow")
        nc.vector.memset(srow[:], 0.0)
        tmpr = wtile([1, 128], f32, "tmpr")
        lam1 = wtile([1, 1], f32, "lam1")
        lam2 = wtile([1, 1], f32, "lam2")
        nc.vector.tensor_mul(tmpr[:], lq1_r[:], lk1_r[:])
        nc.vector.tensor_reduce(lam1[:], tmpr[:], axis=AX.X, op=ALU.add)
        nc.scalar.activation(lam1[:], lam1[:], AF.Exp)
        nc.vector.tensor_mul(tmpr[:], lq2_r[:], lk2_r[:])
        nc.vector.tensor_reduce(lam2[:], tmpr[:], axis=AX.X, op=ALU.add)
        nc.scalar.activation(lam2[:], lam2[:], AF.Exp)
        nc.vector.tensor_sub(srow[:, 0:1], lam1[:], lam2[:])
        nc.vector.tensor_add(srow[:, 0:1], srow[:, 0:1], laminit_c[:])  # lam_full
        nc.scalar.mul(srow[:, 1:2], srow[:, 0:1], -1.0)            # -lam_full
        nc.vector.tensor_copy(srow[:, 2:3], alpha_r[:])
        nc.vector.tensor_copy(srow[:, 3:4], kbias_r[:])

        ps_b = mm_tile(128, 28)
        nc.tensor.matmul(ps_b[:, 0:8], ones_row[:], srow[:],
                         start=True, stop=False)
        nc.tensor.matmul(ps_b[:, 8:18], ones_row[:], kana_r[:],
                         start=False, stop=False)
        nc.tensor.matmul(ps_b[:, 18:28], ones_row[:], kanb_r[:],
                         start=False, stop=True)
        sB = wtile([128, 28], f32, "sB")
        nc.vector.tensor_copy(sB[:], ps_b)
        neglam_c = sB[:, 1:2]
        alpha_c = sB[:, 2:3]
        kbias_c = sB[:, 3:4]

        # ---------- new_e from e (FourierKAN), layout [128, NSUB] ----------
        eT = wtile([128, NSUB], f32, "eT")
        nc.vector.memset(eT[:], 0.0)
        eload = wtile([max(TG_FULL, 1), 128], f32, "eload")
        nc.scalar.dma_start(
            out=eload[:TG_FULL],
            in_=t_e[: TG_FULL * 128].rearrange("(t p) -> t p", p=128))
        pse = mm_tile(128, TG_FULL)
        T(pse, eload[:TG_FULL], ident[:TG_FULL, :TG_FULL])
        nc.vector.tensor_copy(eT[:, :TG_FULL], pse)
        if TG_REM > 0:
            erem = wtile([1, TG_REM], f32, "erem")
            nc.scalar.dma_start(
                out=erem[:],
                in_=t_e[TG_FULL * 128:].rearrange("(x p) -> x p", x=1))
            psr = mm_tile(TG_REM, 1)
            T(psr, erem[:], ident[:1, :1])
            nc.vector.tensor_copy(eT[:TG_REM, TG_FULL:NSUB], psr)

        # Chebyshev recurrence for cos/sin(k*e/pi); theta = e/pi in [0, 0.64]
        s1 = wtile([128, NSUB], f32, "s1")
        nc.scalar.activation(s1[:], eT[:], AF.Sin, scale=1.0 / math.pi)
        c1 = wtile([128, NSUB], f32, "c1")
        nc.vector.tensor_mul(c1[:], s1[:], s1[:])
        nc.scalar.activation(c1[:], c1[:], AF.Sqrt, scale=-1.0, bias=1.0)
        twoc = wtile([128, NSUB], f32, "twoc")
        nc.vector.tensor_add(twoc[:], c1[:], c1[:])

        phi = wtile([128, NSUB], f32, "phi")
        ktmp = wtile([128, NSUB], f32, "ktmp")
        nc.vector.tensor_scalar(phi[:], c1[:], scalar1=sB[:, 8:9], scalar2=None, op0=ALU.mult)
        nc.vector.tensor_scalar(ktmp[:], s1[:], scalar1=sB[:, 18:19],
                                scalar2=None, op0=ALU.mult)
        nc.vector.tensor_add(phi[:], phi[:], ktmp[:])
        cp, sp = c1, s1
        cpp, spp = None, None
        for k in range(2, 11):
            ck = rtile([128, NSUB], f32, "ckt")
            sk = rtile([128, NSUB], f32, "skt")
            nc.vector.tensor_mul(ck[:], twoc[:], cp[:])
            nc.vector.tensor_mul(sk[:], twoc[:], sp[:])
            if k == 2:
                nc.vector.tensor_scalar(ck[:], ck[:], scalar1=one_col[:],
                                        scalar2=None, op0=ALU.subtract)
            else:
                nc.vector.tensor_sub(ck[:], ck[:], cpp[:])
                nc.vector.tensor_sub(sk[:], sk[:], spp[:])
            nc.vector.tensor_scalar(ktmp[:], ck[:],
                                    scalar1=sB[:, 7 + k:8 + k], scalar2=None, op0=ALU.mult)
            nc.vector.tensor_add(phi[:], phi[:], ktmp[:])
            nc.vector.tensor_scalar(ktmp[:], sk[:],
                                    scalar1=sB[:, 17 + k:18 + k], scalar2=None, op0=ALU.mult)
            nc.vector.tensor_add(phi[:], phi[:], ktmp[:])
            cpp, spp = cp, sp
            cp, sp = ck, sk
        ne = wtile([128, NSUB], f32, "ne")
        nc.vector.tensor_scalar(ne[:], phi[:], scalar1=kbias_c, op0=ALU.add,
                                scalar2=alpha_c, op1=ALU.mult)

        # ---------- folded weights (LN affine into projections) ----------
        def fold_w(name, w_sb, g_col):
            t = wtile([128, D], f32, name)
            nc.vector.tensor_scalar(t[:], w_sb[:], scalar1=g_col[:], scalar2=None, op0=ALU.mult)
            return t

        Wk1 = fold_w("Wk1", k1w, mg_c); Wk2 = fold_w("Wk2", k2w, mg_c)
        Wv = fold_w("Wv", vw, mg_c)
        Wq1 = fold_w("Wq1", q1w, mg_c); Wq2 = fold_w("Wq2", q2w, mg_c)
        W1p = fold_w("W1p", f1w, fg_c)
        Wo = wtile([128, D], f32, "Wo")
        nc.vector.tensor_scalar(Wo[:], ow[:], scalar1=ag_c[:], op0=ALU.mult,
                                scalar2=c08_col[:], op1=ALU.mult)

        def fold_b(name, w_sb, beta_col, b_row):
            psb = mm_tile(1, D)
            nc.tensor.matmul(psb, beta_col[:], w_sb[:])
            t = wtile([1, D], f32, name)
            nc.vector.tensor_add(t[:], psb, b_row[:])
            return t

        bk1_r = fold_b("bk1_r", k1w, mb_c, k1b_r)
        bk2_r = fold_b("bk2_r", k2w, mb_c, k2b_r)
        bv_r = fold_b("bv_r", vw, mb_c, vb_r)
        b1p_r = fold_b("b1p_r", f1w, fb_c, f1b_r)
        psq = mm_tile(128, 1)
        nc.tensor.matmul(psq, q1w[:], mb_c[:])
        bq1_c = wtile([128, 1], f32, "bq1_c")
        nc.vector.tensor_add(bq1_c[:], psq, q1b_c[:])
        psq2 = mm_tile(128, 1)
        nc.tensor.matmul(psq2, q2w[:], mb_c[:])
        bq2_c = wtile([128, 1], f32, "bq2_c")
        nc.vector.tensor_add(bq2_c[:], psq2, q2b_c[:])
        pso = mm_tile(1, D)
        nc.tensor.matmul(pso, ab_c[:], ow[:])
        bo_r = wtile([1, D], f32, "bo_r")
        nc.vector.tensor_scalar(bo_r[:], pso, scalar1=c08_col[:1], scalar2=None, op0=ALU.mult)
        nc.vector.tensor_add(bo_r[:], bo_r[:], ob_r[:])

        def bcast_row(name, row_sb):
            psb = mm_tile(128, D)
            nc.tensor.matmul(psb, ones_row[:], row_sb[:])
            t = wtile([128, D], f32, name)
            nc.vector.tensor_copy(t[:], psb)
            return t

        bk1_B = bcast_row("bk1_B", bk1_r)
        bk2_B = bcast_row("bk2_B", bk2_r)
        bv_B = bcast_row("bv_B", bv_r)
        b1p_B = bcast_row("b1p_B", b1p_r)
        bo_B = bcast_row("bo_B", bo_r)

        # ================= phase A: feature encoder =================
        h = [wtile([128, D], f32, f"h{r}") for r in range(NR)]
        h32r = [wtile([128, D], f32r, f"h32r{r}") for r in range(NR)]
        hnT = [wtile([128, 128], f32, f"hnT{r}") for r in range(NR)]
        h1T_all = wtile([128, NLOC], f32, "h1T_all")

        for go, gw in RG:
            xT = rowtmp.tile([128, KX, 512], f32, tag="xT", bufs=2,
                             name=f"xT_{nc.next_id()}")
            for ro, rw in _splits(gw, 128):
                xt = rtile([128, NF], f32, "xt")
                nc.scalar.dma_start(out=xt[:rw],
                                    in_=t_x[go + ro: go + ro + rw, :])
                for kt in range(KX):
                    pst = mm_tile(128, rw)
                    T(pst, xt[:rw, kt * 128:(kt + 1) * 128], ident[:rw, :rw])
                    nc.vector.tensor_copy(xT[:, kt, ro:ro + rw], pst)
            psh1 = p1_tile(gw)
            for kt in range(KX):
                nc.tensor.matmul(psh1, few1[:, kt, :], xT[:, kt, :gw],
                                 start=(kt == 0), stop=(kt == KX - 1))
            nc.scalar.activation(h1T_all[:, go:go + gw], psh1, AF.Relu,
                                 bias=feb1_c[:])

        for r, (ro, rw) in enumerate(ROWS):
            psh = mm_tile(rw, D)
            nc.tensor.matmul(psh, h1T_all[:, ro:ro + rw], few2[:])
            nc.vector.tensor_add(h[r][:rw], psh, feb2_B[:rw])
            nc.vector.tensor_add(h32r[r][:rw], psh, feb2_B[:rw])

        # ================= phase B: LN + k/v projections + gram =================
        def layer_norm(src_ap, rw, out_ap):
            stats = rtile([128, 6], f32, "stats")
            nc.vector.bn_stats(stats[:rw], src_ap)
            mv = rtile([128, 2], f32, "mv")
            nc.vector.bn_aggr(mv[:rw], stats[:rw])
            rs = rtile([128, 1], f32, "rs")
            nc.scalar.activation(rs[:rw], mv[:rw, 1:2], AF.Sqrt,
                                 bias=eps_col[:rw])
            nc.vector.reciprocal(rs[:rw], rs[:rw])
            nc.vector.tensor_scalar(out_ap, src_ap, scalar1=mv[:rw, 0:1],
                                    op0=ALU.subtract, scalar2=rs[:rw],
                                    op1=ALU.mult)

        gram = wtile([128, 2 * D], f32, "gram")
        for r, (ro, rw) in enumerate(ROWS):
            hn = rtile([128, D], f32, "hn")
            layer_norm(h[r][:rw], rw, hn[:rw])
            psT = mm_tile(128, rw)
            T(psT, hn[:rw], ident[:rw, :rw])
            nc.vector.tensor_copy(hnT[r][:, :rw], psT)

            k1t = rtile([128, D], f32, "k1t")
            k2t = rtile([128, D], f32, "k2t")
            vt = rtile([128, D], f32, "vt")
            for dst, W, bB in ((k1t, Wk1, bk1_B), (k2t, Wk2, bk2_B),
                               (vt, Wv, bv_B)):
                psp = mm_tile(rw, D)
                nc.tensor.matmul(psp, hnT[r][:, :rw], W[:])
                nc.vector.tensor_add(dst[:rw], psp, bB[:rw])
            psg1 = mm_tile(D, D)
            nc.tensor.matmul(psg1, k1t[:rw], vt[:rw])
            psg2 = mm_tile(D, D)
            nc.tensor.matmul(psg2, k2t[:rw], vt[:rw])
            if r == 0:
                nc.vector.tensor_copy(gram[:, :D], psg1)
                nc.vector.tensor_copy(gram[:, D:], psg2)
            else:
                nc.vector.tensor_add(gram[:, :D], gram[:, :D], psg1)
                nc.vector.tensor_add(gram[:, D:], gram[:, D:], psg2)

        # ---------------- gram all-reduce ----------------
        gr_in = dram.tile([128, 2 * D], f32, tag="gr_in", name="gr_in")
        gr_out = dram.tile([128, 2 * D], f32, tag="gr_out", name="gr_out",
                           addr_space=shared_space)
        nc.gpsimd.dma_start(out=gr_in[:], in_=gram[:])
        nc.gpsimd.collective_compute("AllReduce", ALU.add, replica_groups=rg,
                                     ins=[gr_in.opt()], outs=[gr_out.opt()])

        def emit_watt():
            # gram AR completed long ago; safe to consume without stalling
            kv = wtile([128, 2 * D], f32, "kv")
            nc.scalar.dma_start(out=kv[:], in_=gr_out[:])
            psq1T = mm_tile(128, 128)
            T(psq1T, Wq1[:], ident[:])
            Wq1T = wtile([128, D], f32, "Wq1T")
            nc.vector.tensor_copy(Wq1T[:], psq1T)
            psq2T = mm_tile(128, 128)
            T(psq2T, Wq2[:], ident[:])
            Wq2T = wtile([128, D], f32, "Wq2T")
            nc.vector.tensor_copy(Wq2T[:], psq2T)

            ps_w1e = mm_tile(D, D)
            nc.tensor.matmul(ps_w1e, Wq1T[:], kv[:, :D])
            ps_w2e = mm_tile(D, D)
            nc.tensor.matmul(ps_w2e, Wq2T[:], kv[:, D:])
            Watt = wtile([128, D], f32, "Watt")
            nc.vector.tensor_scalar(Watt[:], ps_w2e, scalar1=neglam_c,
                                    scalar2=None, op0=ALU.mult)
            nc.vector.tensor_add(Watt[:], Watt[:], ps_w1e)

            ps_b1 = mm_tile(1, D)
            nc.tensor.matmul(ps_b1, bq1_c[:], kv[:, :D])
            ps_b2 = mm_tile(1, D)
            nc.tensor.matmul(ps_b2, bq2_c[:], kv[:, D:])
            batt_r = wtile([1, D], f32, "batt_r")
            nc.vector.tensor_scalar(batt_r[:], ps_b2, scalar1=neglam_c[:1],
                                    scalar2=None, op0=ALU.mult)
            nc.vector.tensor_add(batt_r[:], batt_r[:], ps_b1)
            batt_B = bcast_row("batt_B", batt_r)
            return Watt, batt_B

        # ================= spectral pipeline =================
        NLOC_PAD = _ceil_div(NLOC, 16) * 16
        N_PAD = NSUB * 128
        u16 = dram.tile([NLOC_PAD, N_PAD], bf16, tag="u16", name="u16")
        p1_in, p1_out = [], []
        for c, (co, cw) in enumerate(CHUNKS):
            p1_in.append(dram.tile([128, cw], bf16, tag=f"p1in{c}",
                                   name=f"p1in{c}"))
            p1_out.append(dram.tile([128, cw], bf16, tag=f"p1out{c}",
                                    name=f"p1out{c}", addr_space=shared_space))

        # zero-fill the padded regions of u16 (read by transposed loads)
        zpad = wtile([128, 1024], bf16, "zpad")
        nc.vector.memset(zpad[:], 0.0)
        if NLOC_PAD > NLOC:
            pr = NLOC_PAD - NLOC
            for jo, jw in _splits(N_PAD, 1024):
                nc.gpsimd.dma_start(out=u16[NLOC:NLOC_PAD, jo:jo + jw],
                                    in_=zpad[:pr, :jw])
        if N_PAD > N:
            pw = N_PAD - N
            for io, iw2 in _splits(NLOC, 128):
                nc.gpsimd.dma_start(out=u16[io:io + iw2, N:N_PAD],
                                    in_=zpad[:iw2, :pw])

        henc = wtile([128, NLOC], f32, "henc")

        def emit_chunk_pass1(c):
            co, cw = CHUNKS[c]
            utx = stg.tile([128, CHUNK], bf16, tag="utx",
                           name=f"utx{c}")[:, :cw]
            for b, (bo, bw) in enumerate(_splits(cw, 512)):
                ps1 = p1_tile(bw)
                for r, (ro, rw) in enumerate(ROWS):
                    ut = ustream.tile([128, 512], f32r, tag="u",
                                      name=f"u{c}_{b}_{r}")[:rw, :bw]
                    nc.scalar.dma_start(
                        out=ut, in_=t_u[ro:ro + rw, co + bo:co + bo + bw])
                    nc.tensor.matmul(ps1, h32r[r][:rw], ut,
                                     start=(r == 0), stop=(r == NR - 1))
                    nc.gpsimd.dma_start(
                        out=u16[ro:ro + rw, co + bo:co + bo + bw],
                        in_=ut.bitcast(f32))
                nc.vector.tensor_copy(utx[:, bo:bo + bw], ps1)
            nc.gpsimd.dma_start(out=p1_in[c][:], in_=utx)
            nc.gpsimd.collective_compute(
                "AllReduce", ALU.add, replica_groups=rg,
                ins=[p1_in[c].opt()], outs=[p1_out[c].opt()])

        def emit_chunk_pass2(c):
            co, cw = CHUNKS[c]
            subs = _splits(cw, 128)
            # uT transposed streams first (not gated on the all-reduce);
            # alternate HWDGE queues (SP/ACT) to double xbar throughput
            uTtiles = []
            for t, (so, sw) in enumerate(subs):
                uTtile = uTt.tile([128, NLOC_PAD], bf16, tag="uTt",
                                  name=f"uTt{c}_{t}")
                nc.sync.dma_start(out=uTtile[:],
                                   in_=u16[:, co + so:co + so + 128],
                                   transpose=True)
                uTtiles.append(uTtile)
            z = zp.tile([128, NSUB_C, D], bf16, tag="z", name=f"z{c}")
            for t, (so, sw) in enumerate(subs):
                zr = rtile([128, 128], bf16, "zr")
                if sw % 128 == 0:
                    nc.sync.dma_start(out=zr[:sw, :],
                                      in_=p1_out[c][:, so:so + sw],
                                      transpose=True)
                else:
                    nc.scalar.dma_start(
                        out=zr[:sw, :],
                        in_=p1_out[c][:, so:so + sw].rearrange("a b -> b a"))
                gidx = (co + so) // 128
                nc.vector.tensor_scalar(z[:sw, t, :], zr[:sw, :],
                                        scalar1=ne[:sw, gidx:gidx + 1],
                                        scalar2=None, op0=ALU.mult)
            ps2 = [p2_tile(iw) for io, iw in IBLK]
            for t, (so, sw) in enumerate(subs):
                for ib, (io, iw) in enumerate(IBLK):
                    nc.tensor.matmul(ps2[ib], z[:sw, t, :],
                                     uTtiles[t][:sw, io:io + iw],
                                     start=(t == 0), stop=(t == len(subs) - 1))
            for ib, (io, iw) in enumerate(IBLK):
                if c == 0:
                    nc.vector.tensor_copy(henc[:, io:io + iw], ps2[ib])
                else:
                    nc.vector.tensor_add(henc[:, io:io + iw],
                                         henc[:, io:io + iw], ps2[ib])

        # ========== chunk pipeline: pass2 three stages behind pass1 ==========
        NCH = len(CHUNKS)
        DEPTH = min(3, NCH - 1)
        for c in range(NCH):
            emit_chunk_pass1(c)
            if c >= DEPTH:
                emit_chunk_pass2(c - DEPTH)

        # == attention (gram AR completed during early chunks) ==
        Watt, batt_B = emit_watt()
        ha = [wtile([128, D], f32, f"ha{r}") for r in range(NR)]
        for r, (ro, rw) in enumerate(ROWS):
            pss = mm_tile(rw, D)
            nc.tensor.matmul(pss, hnT[r][:, :rw], Watt[:])
            s_sb = rtile([128, D], f32, "s_sb")
            nc.vector.tensor_add(s_sb[:rw], pss, batt_B[:rw])
            layer_norm(s_sb[:rw], rw, s_sb[:rw])
            psT = mm_tile(128, rw)
            T(psT, s_sb[:rw], ident[:rw, :rw])
            sT = rtile([128, 128], f32, "sT")
            nc.vector.tensor_copy(sT[:, :rw], psT)
            psa = mm_tile(rw, D)
            nc.tensor.matmul(psa, sT[:, :rw], Wo[:])
            att = rtile([128, D], f32, "att")
            nc.vector.tensor_add(att[:rw], psa, bo_B[:rw])
            nc.vector.tensor_add(ha[r][:rw], h[r][:rw], att[:rw])

        for c in range(max(0, NCH - DEPTH), NCH):
            emit_chunk_pass2(c)

        # ================= residual + FFN =================
        for r, (ro, rw) in enumerate(ROWS):
            psb = mm_tile(rw, D)
            T(psb, henc[:, ro:ro + rw], ident[:, :])
            mh = rtile([128, D], f32, "mh")
            nc.vector.tensor_add(mh[:rw], ha[r][:rw], psb)
            fh = rtile([128, D], f32, "fh")
            layer_norm(mh[:rw], rw, fh[:rw])
            psT = mm_tile(128, rw)
            T(psT, fh[:rw], ident[:rw, :rw])
            fT = rtile([128, 128], f32, "fT")
            nc.vector.tensor_copy(fT[:, :rw], psT)
            psg = mm_tile(rw, D)
            nc.tensor.matmul(psg, fT[:, :rw], W1p[:])
            gl = rtile([128, D], f32, "gl")
            nc.vector.tensor_add(gl[:rw], psg, b1p_B[:rw])
            if sim_gelu:
                # tanh-approx gelu (CoreSim lacks Gelu); HW build uses AF.Gelu
                x3 = rtile([128, D], f32, "x3")
                nc.vector.tensor_mul(x3[:rw], gl[:rw], gl[:rw])
                nc.vector.tensor_mul(x3[:rw], x3[:rw], gl[:rw])
                nc.vector.tensor_scalar(x3[:rw], x3[:rw], scalar1=0.044715,
                                        scalar2=None, op0=ALU.mult)
                nc.vector.tensor_add(x3[:rw], x3[:rw], gl[:rw])
                nc.scalar.activation(x3[:rw], x3[:rw], AF.Tanh,
                                     scale=math.sqrt(2.0 / math.pi))
                nc.vector.tensor_scalar(x3[:rw], x3[:rw], scalar1=1.0,
                                        scalar2=0.5, op0=ALU.add, op1=ALU.mult)
                nc.vector.tensor_mul(gl[:rw], gl[:rw], x3[:rw])
            else:
                nc.scalar.activation(gl[:rw], gl[:rw], AF.Gelu)
            psT2 = mm_tile(128, rw)
            T(psT2, gl[:rw], ident[:rw, :rw])
            gT = rtile([128, 128], f32, "gT")
            nc.vector.tensor_copy(gT[:, :rw], psT2)
            pso2 = mm_tile(rw, D)
            nc.tensor.matmul(pso2, gT[:, :rw], f2w[:])
            outp = rtile([128, D], f32, "outp")
            nc.vector.tensor_add(outp[:rw], pso2, mh[:rw])
            nc.vector.tensor_add(outp[:rw], outp[:rw], f2b_B[:rw])
            nc.scalar.dma_start(out=t_out[ro:ro + rw, :], in_=outp[:rw])

    nc.compile()
    return nc


# ==================== host-side entry point ====================

_CACHED = {}


def _get_nc(N=N_FULL, NF=NF_FULL, CORES=CORES_FULL, CHUNK=CHUNK_FULL):
    key = (N, NF, CORES, CHUNK)
    if key not in _CACHED:
        _CACHED[key] = build_kernel(N, NF, CORES, CHUNK)
    return _CACHED[key]


def make_in_maps(inputs, N, CORES):
    NLOC = N // CORES
    full = {k: np.ascontiguousarray(np.asarray(v, dtype=np.float32))
            for k, v in inputs.items()}
    in_maps = []
    for c in range(CORES):
        m = {}
        for k, v in full.items():
            if k in ("x", "u"):
                m[k] = np.ascontiguousarray(v[c * NLOC:(c + 1) * NLOC])
            else:
                m[k] = v
        in_maps.append(m)
    return in_maps


def kernel(**inputs):
    from concourse import bass_utils

    nc = _get_nc()
    in_maps = make_in_maps(inputs, N_FULL, CORES_FULL)
    res = bass_utils.run_bass_kernel_spmd(nc, in_maps,
                                          core_ids=list(range(CORES_FULL)))
    out = np.concatenate([res.results[c]["out"] for c in range(CORES_FULL)],
                         axis=0)
    return out.astype(np.float32)


if __name__ == "__main__":
    build_kernel()
    print("build ok")



# revision 5
# speedup vs baseline: 2.9208x; 2.9208x over previous
"""Trainium2 Bass kernel for nn_NoFoDifformer_FourierKAN (8-core SPMD).

Sharding: u and nodes row-wise across 8 cores (1250 rows each). The [d,d]
K^T V Gram matrices and the chunked u^T h partial sums are all-reduced;
small weights are replicated; per-core row-shard outputs are concatenated
on the host.

The host pre-shards u into TWO bf16 tensors per core: u16 = u[rows,:] for
pass-1 (utx partials) and ut16 = u[rows,:].T for pass-2, both zero-padded
to 79*128 columns/rows. This removes the on-device transpose round-trip
entirely; the device streams each tensor once with large contiguous DMAs.
x is host-pre-transposed so the feature encoder needs no PE transposes of
x. LayerNorm affine params are folded into downstream projection weights.
"""

import math
from contextlib import ExitStack

import numpy as np

N_FULL = 10000
NF_FULL = 512
D = 128
CORES_FULL = 8
N_PAD = 10112                  # 79 * 128
LAMBDA_INIT = 0.2
CHUNK_LIST = [2048, 2048, 2048, 2048, 1408, 512]   # sums to N_PAD


def _splits(total, step):
    return [(o, min(step, total - o)) for o in range(0, total, step)]


def build_kernel(N=N_FULL, NF=NF_FULL, CORES=CORES_FULL, sim_gelu=False):
    import concourse.bacc as bacc
    import concourse.tile as tile
    from concourse import mybir
    from concourse.masks import make_identity

    dt = mybir.dt
    f32 = dt.float32
    bf16 = dt.bfloat16
    AF = mybir.ActivationFunctionType
    ALU = mybir.AluOpType
    AX = mybir.AxisListType

    NLOC = N // CORES
    assert NLOC * CORES == N
    ROWS = _splits(NLOC, 128)          # per-core row tiles (i)
    NR = len(ROWS)
    KX = NF // 128                     # x feature k-tiles
    assert KX * 128 == NF
    NSUB = N_PAD // 128                # 79 j-subtiles
    chunks, off = [], 0
    for cw in CHUNK_LIST:
        chunks.append((off, cw))
        off += cw
    assert off == N_PAD
    NCH = len(chunks)
    IBLK = _splits(NLOC, 512)          # pass-2 output i blocks
    TG_FULL = N // 128                 # full 128-wide e subtiles
    TG_REM = N - TG_FULL * 128
    rg = [list(range(CORES))]
    shared_space = "Shared" if CORES > 4 else "Local"

    nc = bacc.Bacc("TRN2", target_bir_lowering=False, debug=False,
                   num_devices=CORES)

    # ---------------- DRAM I/O ----------------
    def din(name, shape):
        return nc.dram_tensor(name, list(shape), f32, kind="ExternalInput")

    t_xT = din("xT", (NF, NLOC))
    t_u16 = nc.dram_tensor("u16", [NLOC, N_PAD], bf16, kind="ExternalInput")
    t_ut16 = nc.dram_tensor("ut16", [N_PAD, NLOC], bf16, kind="ExternalInput")
    t_e = din("e", (N,))
    t_few1 = din("fe_w1", (NF, D)); t_feb1 = din("fe_b1", (D,))
    t_few2 = din("fe_w2", (D, D)); t_feb2 = din("fe_b2", (D,))
    t_kana = din("kan_a", (10,)); t_kanb = din("kan_b", (10,))
    t_kanbias = din("kan_bias", (1,)); t_alpha = din("alpha_w", (1, 1))
    t_mg = din("mha_ln_g", (D,)); t_mb = din("mha_ln_b", (D,))
    t_fg = din("ffn_ln_g", (D,)); t_fb = din("ffn_ln_b", (D,))
    t_q1w = din("q1_w", (D, D)); t_q1b = din("q1_b", (D,))
    t_k1w = din("k1_w", (D, D)); t_k1b = din("k1_b", (D,))
    t_q2w = din("q2_w", (D, D)); t_q2b = din("q2_b", (D,))
    t_k2w = din("k2_w", (D, D)); t_k2b = din("k2_b", (D,))
    t_vw = din("v_w", (D, D)); t_vb = din("v_b", (D,))
    t_ag = din("attn_ln_g", (D,)); t_ab = din("attn_ln_b", (D,))
    t_ow = din("out_w", (D, D)); t_ob = din("out_b", (D,))
    t_lq1 = din("lq1", (D,)); t_lk1 = din("lk1", (D,))
    t_lq2 = din("lq2", (D,)); t_lk2 = din("lk2", (D,))
    t_f1w = din("ffn1_w", (D, D)); t_f1b = din("ffn1_b", (D,))
    t_f2w = din("ffn2_w", (D, D)); t_f2b = din("ffn2_b", (D,))
    t_out = nc.dram_tensor("out", [NLOC, D], f32, kind="ExternalOutput")

    with tile.TileContext(nc) as tc, ExitStack() as ctx:
        wpool = ctx.enter_context(tc.tile_pool(name="wpool", bufs=1))
        rowtmp = ctx.enter_context(tc.tile_pool(name="rowtmp", bufs=3))
        ustream = ctx.enter_context(tc.tile_pool(name="ustream", bufs=6))
        utstream = ctx.enter_context(tc.tile_pool(name="utstream", bufs=16))
        utxst = ctx.enter_context(tc.tile_pool(name="utxst", bufs=2))
        utxrd = ctx.enter_context(tc.tile_pool(name="utxrd", bufs=2))
        zpool = ctx.enter_context(tc.tile_pool(name="zpool", bufs=4))
        xtp = ctx.enter_context(tc.tile_pool(name="xtp", bufs=4))
        dram = ctx.enter_context(tc.tile_pool(name="dram", bufs=1, space="DRAM"))
        ps_p1 = ctx.enter_context(tc.tile_pool(name="ps_p1", bufs=4, space="PSUM"))
        ps_p2 = ctx.enter_context(tc.tile_pool(name="ps_p2", bufs=3, space="PSUM"))
        ps_mm = ctx.enter_context(tc.tile_pool(name="ps_mm", bufs=1, space="PSUM"))

        def p1_tile():
            return ps_p1.tile([128, 512], f32, tag="p1",
                              name=f"p1_{nc.next_id()}")

        def p2_tile():
            return ps_p2.tile([128, 512], f32, tag="p2",
                              name=f"p2_{nc.next_id()}")

        def mm_tile():
            return ps_mm.tile([128, 512], f32, tag="mm",
                              name=f"mm_{nc.next_id()}")

        def mmz_tile():
            return ps_mm.tile([128, 512], bf16, tag="mm",
                              name=f"mmz_{nc.next_id()}")

        def wtile(shape, dtype, name):
            return wpool.tile(shape, dtype, tag=name, name=name)

        def rtile(shape, dtype, tag):
            return rowtmp.tile(shape, dtype, tag=tag,
                               name=f"{tag}_{nc.next_id()}")

        def T(out_psum, in_sbuf, identity):
            nc.tensor.matmul(out_psum, in_sbuf, identity, is_transpose=True)

        # ================= constants & weights =================
        ident = wtile([128, 128], f32, "ident")
        make_identity(nc, ident[:])
        identb = wtile([128, 128], bf16, "identb")
        make_identity(nc, identb[:])

        ones_row = wtile([1, 128], f32, "ones_row")
        nc.vector.memset(ones_row[:], 1.0)
        eps_col = wtile([128, 1], f32, "eps_col")
        nc.vector.memset(eps_col[:], 1e-5)
        c08_col = wtile([128, 1], f32, "c08_col")
        nc.vector.memset(c08_col[:], 1.0 - LAMBDA_INIT)
        one_col = wtile([128, 1], f32, "one_col")
        nc.vector.memset(one_col[:], 1.0)
        laminit_c = wtile([1, 1], f32, "laminit_c")
        nc.vector.memset(laminit_c[:], LAMBDA_INIT)

        def ldw(name, dram_t, shape, rearr=None, **kw):
            t = wtile(shape, f32, name)
            src = dram_t[:] if rearr is None else dram_t[:].rearrange(rearr, **kw)
            nc.scalar.dma_start(out=t[:], in_=src)
            return t

        few1 = ldw("few1", t_few1, [128, KX, D], "(t p) d -> p t d", p=128)
        few2 = ldw("few2", t_few2, [128, D])
        q1w = ldw("q1w", t_q1w, [128, D])
        k1w = ldw("k1w", t_k1w, [128, D])
        q2w = ldw("q2w", t_q2w, [128, D])
        k2w = ldw("k2w", t_k2w, [128, D])
        vw = ldw("vw", t_vw, [128, D])
        ow = ldw("ow", t_ow, [128, D])
        f1w = ldw("f1w", t_f1w, [128, D])
        f2w = ldw("f2w", t_f2w, [128, D])

        def ldcol(name, dram_t):
            t = wtile([128, 1], f32, name)
            nc.scalar.dma_start(out=t[:],
                                in_=dram_t[:].rearrange("(p x) -> p x", x=1))
            return t

        feb1_c = ldcol("feb1_c", t_feb1)
        feb2_c = ldcol("feb2_c", t_feb2)
        mg_c = ldcol("mg_c", t_mg); mb_c = ldcol("mb_c", t_mb)
        fg_c = ldcol("fg_c", t_fg); fb_c = ldcol("fb_c", t_fb)
        ag_c = ldcol("ag_c", t_ag); ab_c = ldcol("ab_c", t_ab)
        q1b_c = ldcol("q1b_c", t_q1b); q2b_c = ldcol("q2b_c", t_q2b)

        def ldrow(name, dram_t, w=128):
            t = wtile([1, w], f32, name)
            nc.scalar.dma_start(out=t[:],
                                in_=dram_t[:].rearrange("(x p) -> x p", x=1))
            return t

        k1b_r = ldrow("k1b_r", t_k1b); k2b_r = ldrow("k2b_r", t_k2b)
        vb_r = ldrow("vb_r", t_vb); ob_r = ldrow("ob_r", t_ob)
        f1b_r = ldrow("f1b_r", t_f1b)
        lq1_r = ldrow("lq1_r", t_lq1); lk1_r = ldrow("lk1_r", t_lk1)
        lq2_r = ldrow("lq2_r", t_lq2); lk2_r = ldrow("lk2_r", t_lk2)
        kana_r = ldrow("kana_r", t_kana, 10)
        kanb_r = ldrow("kanb_r", t_kanb, 10)
        kbias_r = ldrow("kbias_r", t_kanbias, 1)
        alpha_r = wtile([1, 1], f32, "alpha_r")
        nc.scalar.dma_start(out=alpha_r[:], in_=t_alpha[:])

        def ldbcast(name, dram_t):
            t = wtile([128, D], f32, name)
            nc.scalar.dma_start(out=t[:], in_=dram_t[:].partition_broadcast(128))
            return t

        f2b_B = ldbcast("f2b_B", t_f2b)

        # ---------- scalars: lambda ----------
        srow = wtile([1, 8], f32, "srow")
        nc.vector.memset(srow[:], 0.0)
        tmpr = wtile([1, 128], f32, "tmpr")
        lam1 = wtile([1, 1], f32, "lam1")
        lam2 = wtile([1, 1], f32, "lam2")
        nc.vector.tensor_mul(tmpr[:], lq1_r[:], lk1_r[:])
        nc.vector.tensor_reduce(lam1[:], tmpr[:], axis=AX.X, op=ALU.add)
        nc.scalar.activation(lam1[:], lam1[:], AF.Exp)
        nc.vector.tensor_mul(tmpr[:], lq2_r[:], lk2_r[:])
        nc.vector.tensor_reduce(lam2[:], tmpr[:], axis=AX.X, op=ALU.add)
        nc.scalar.activation(lam2[:], lam2[:], AF.Exp)
        nc.vector.tensor_sub(srow[:, 0:1], lam1[:], lam2[:])
        nc.vector.tensor_add(srow[:, 0:1], srow[:, 0:1], laminit_c[:])  # lam_full
        nc.scalar.mul(srow[:, 1:2], srow[:, 0:1], -1.0)            # -lam_full
        nc.vector.tensor_copy(srow[:, 2:3], alpha_r[:])
        nc.vector.tensor_copy(srow[:, 3:4], kbias_r[:])

        ps_b = mm_tile()[:, :28]
        nc.tensor.matmul(ps_b[:, 0:8], ones_row[:], srow[:],
                         start=True, stop=False)
        nc.tensor.matmul(ps_b[:, 8:18], ones_row[:], kana_r[:],
                         start=False, stop=False)
        nc.tensor.matmul(ps_b[:, 18:28], ones_row[:], kanb_r[:],
                         start=False, stop=True)
        sB = wtile([128, 28], f32, "sB")
        nc.vector.tensor_copy(sB[:], ps_b)
        neglam_c = sB[:, 1:2]
        alpha_c = sB[:, 2:3]
        kbias_c = sB[:, 3:4]

        # ---------- new_e from e (FourierKAN), layout [128, NSUB] ----------
        eT = wtile([128, NSUB], f32, "eT")
        nc.vector.memset(eT[:], 0.0)
        eload = wtile([max(TG_FULL, 1), 128], f32, "eload")
        nc.scalar.dma_start(
            out=eload[:TG_FULL],
            in_=t_e[: TG_FULL * 128].rearrange("(t p) -> t p", p=128))
        pse = mm_tile()[:, :TG_FULL]
        T(pse, eload[:TG_FULL], ident[:TG_FULL, :TG_FULL])
        nc.vector.tensor_copy(eT[:, :TG_FULL], pse)
        if TG_REM > 0:
            erem = wtile([1, TG_REM], f32, "erem")
            nc.scalar.dma_start(
                out=erem[:],
                in_=t_e[TG_FULL * 128:].rearrange("(x p) -> x p", x=1))
            psr = mm_tile()[:TG_REM, :1]
            T(psr, erem[:], ident[:1, :1])
            nc.vector.tensor_copy(eT[:TG_REM, TG_FULL:TG_FULL + 1], psr)

        # Chebyshev recurrence for cos/sin(k*e/pi); theta = e/pi in [0, 0.64]
        s1 = wtile([128, NSUB], f32, "s1")
        nc.scalar.activation(s1[:], eT[:], AF.Sin, scale=1.0 / math.pi)
        c1 = wtile([128, NSUB], f32, "c1")
        nc.vector.tensor_mul(c1[:], s1[:], s1[:])
        nc.scalar.activation(c1[:], c1[:], AF.Sqrt, scale=-1.0, bias=1.0)
        twoc = wtile([128, NSUB], f32, "twoc")
        nc.vector.tensor_add(twoc[:], c1[:], c1[:])

        phi = wtile([128, NSUB], f32, "phi")
        ktmp = wtile([128, NSUB], f32, "ktmp")
        nc.vector.tensor_scalar(phi[:], c1[:], scalar1=sB[:, 8:9],
                                scalar2=None, op0=ALU.mult)
        nc.vector.tensor_scalar(ktmp[:], s1[:], scalar1=sB[:, 18:19],
                                scalar2=None, op0=ALU.mult)
        nc.vector.tensor_add(phi[:], phi[:], ktmp[:])
        cp, sp = c1, s1
        cpp, spp = None, None
        for k in range(2, 11):
            ck = rtile([128, NSUB], f32, "ckt")
            sk = rtile([128, NSUB], f32, "skt")
            nc.vector.tensor_mul(ck[:], twoc[:], cp[:])
            nc.vector.tensor_mul(sk[:], twoc[:], sp[:])
            if k == 2:
                nc.vector.tensor_scalar(ck[:], ck[:], scalar1=one_col[:],
                                        scalar2=None, op0=ALU.subtract)
            else:
                nc.vector.tensor_sub(ck[:], ck[:], cpp[:])
                nc.vector.tensor_sub(sk[:], sk[:], spp[:])
            nc.vector.tensor_scalar(ktmp[:], ck[:],
                                    scalar1=sB[:, 7 + k:8 + k],
                                    scalar2=None, op0=ALU.mult)
            nc.vector.tensor_add(phi[:], phi[:], ktmp[:])
            nc.vector.tensor_scalar(ktmp[:], sk[:],
                                    scalar1=sB[:, 17 + k:18 + k],
                                    scalar2=None, op0=ALU.mult)
            nc.vector.tensor_add(phi[:], phi[:], ktmp[:])
            cpp, spp = cp, sp
            cp, sp = ck, sk
        ne = wtile([128, NSUB], f32, "ne")
        nc.vector.tensor_scalar(ne[:], phi[:], scalar1=kbias_c, op0=ALU.add,
                                scalar2=alpha_c, op1=ALU.mult)

        # ---------- folded weights (LN affine into projections) ----------
        def fold_w(name, w_sb, g_col):
            t = wtile([128, D], f32, name)
            nc.vector.tensor_scalar(t[:], w_sb[:], scalar1=g_col[:],
                                    scalar2=None, op0=ALU.mult)
            return t

        Wk1 = fold_w("Wk1", k1w, mg_c); Wk2 = fold_w("Wk2", k2w, mg_c)
        Wv = fold_w("Wv", vw, mg_c)
        Wq1 = fold_w("Wq1", q1w, mg_c); Wq2 = fold_w("Wq2", q2w, mg_c)
        W1p = fold_w("W1p", f1w, fg_c)
        Wo = wtile([128, D], f32, "Wo")
        nc.vector.tensor_scalar(Wo[:], ow[:], scalar1=ag_c[:], op0=ALU.mult,
                                scalar2=c08_col[:], op1=ALU.mult)

        def fold_b(name, w_sb, beta_col, b_row):
            psb = mm_tile()[:1, :D]
            nc.tensor.matmul(psb, beta_col[:], w_sb[:])
            t = wtile([1, D], f32, name)
            nc.vector.tensor_add(t[:], psb, b_row[:])
            return t

        bk1_r = fold_b("bk1_r", k1w, mb_c, k1b_r)
        bk2_r = fold_b("bk2_r", k2w, mb_c, k2b_r)
        bv_r = fold_b("bv_r", vw, mb_c, vb_r)
        b1p_r = fold_b("b1p_r", f1w, fb_c, f1b_r)
        psq = mm_tile()[:, :1]
        nc.tensor.matmul(psq, q1w[:], mb_c[:])
        bq1_c = wtile([128, 1], f32, "bq1_c")
        nc.vector.tensor_add(bq1_c[:], psq, q1b_c[:])
        psq2 = mm_tile()[:, :1]
        nc.tensor.matmul(psq2, q2w[:], mb_c[:])
        bq2_c = wtile([128, 1], f32, "bq2_c")
        nc.vector.tensor_add(bq2_c[:], psq2, q2b_c[:])
        pso = mm_tile()[:1, :D]
        nc.tensor.matmul(pso, ab_c[:], ow[:])
        bo_r = wtile([1, D], f32, "bo_r")
        nc.vector.tensor_scalar(bo_r[:], pso, scalar1=c08_col[:1],
                                scalar2=None, op0=ALU.mult)
        nc.vector.tensor_add(bo_r[:], bo_r[:], ob_r[:])

        def bcast_row(name, row_sb):
            psb = mm_tile()[:, :D]
            nc.tensor.matmul(psb, ones_row[:], row_sb[:])
            t = wtile([128, D], f32, name)
            nc.vector.tensor_copy(t[:], psb)
            return t

        bk1_B = bcast_row("bk1_B", bk1_r)
        bk2_B = bcast_row("bk2_B", bk2_r)
        bv_B = bcast_row("bv_B", bv_r)
        b1p_B = bcast_row("b1p_B", b1p_r)
        bo_B = bcast_row("bo_B", bo_r)

        # ================= phase A: feature encoder (host-transposed x) ====
        xT_t = []
        for kt in range(KX):
            t = xtp.tile([128, NLOC], f32, tag=f"xt{kt}", name=f"xt{kt}",
                         bufs=1)
            nc.sync.dma_start(out=t[:],
                              in_=t_xT[kt * 128:(kt + 1) * 128, :])
            xT_t.append(t)

        h1T = wtile([128, NLOC], f32, "h1T")
        for io, iw in IBLK:
            psh = p1_tile()[:, :iw]
            for kt in range(KX):
                nc.tensor.matmul(psh, few1[:, kt, :], xT_t[kt][:, io:io + iw],
                                 start=(kt == 0), stop=(kt == KX - 1))
            nc.scalar.activation(h1T[:, io:io + iw], psh, AF.Relu,
                                 bias=feb1_c[:])
        hT = wtile([128, NLOC], f32, "hT")
        for io, iw in IBLK:
            psh = p1_tile()[:, :iw]
            nc.tensor.matmul(psh, few2[:], h1T[:, io:io + iw])
            nc.scalar.add(hT[:, io:io + iw], psh, feb2_c[:])

        h = [wtile([128, D], f32, f"h{r}") for r in range(NR)]
        h16 = [wtile([128, D], bf16, f"h16_{r}") for r in range(NR)]
        hnT = [wtile([128, 128], f32, f"hnT{r}") for r in range(NR)]
        for r, (ro, rw) in enumerate(ROWS):
            pst = p2_tile()[:rw, :D]
            T(pst, hT[:, ro:ro + rw], ident[:])
            nc.vector.tensor_copy(h[r][:rw], pst)
            nc.vector.tensor_copy(h16[r][:rw], pst)

        # ================= phase B: LN + k/v projections + gram =================
        def layer_norm(src_ap, rw, out_ap):
            stats = rtile([128, 6], f32, "stats")
            nc.vector.bn_stats(stats[:rw], src_ap)
            mv = rtile([128, 2], f32, "mv")
            nc.vector.bn_aggr(mv[:rw], stats[:rw])
            rs = rtile([128, 1], f32, "rs")
            nc.scalar.activation(rs[:rw], mv[:rw, 1:2], AF.Sqrt,
                                 bias=eps_col[:rw])
            nc.vector.reciprocal(rs[:rw], rs[:rw])
            nc.vector.tensor_scalar(out_ap, src_ap, scalar1=mv[:rw, 0:1],
                                    op0=ALU.subtract, scalar2=rs[:rw],
                                    op1=ALU.mult)

        gram_ps = ps_mm.tile([128, 512], f32, tag="mm", name="gram_ps")
        for r, (ro, rw) in enumerate(ROWS):
            hn = rtile([128, D], f32, "hn")
            layer_norm(h[r][:rw], rw, hn[:rw])
            psT = p2_tile()[:, :rw]
            T(psT, hn[:rw], ident[:rw, :rw])
            nc.vector.tensor_copy(hnT[r][:, :rw], psT)

            k1t = rtile([128, D], f32, "k1t")
            k2t = rtile([128, D], f32, "k2t")
            vt = rtile([128, D], f32, "vt")
            for dst, W, bB in ((k1t, Wk1, bk1_B), (k2t, Wk2, bk2_B),
                               (vt, Wv, bv_B)):
                psp = p1_tile()[:rw, :D]
                nc.tensor.matmul(psp, hnT[r][:, :rw], W[:])
                nc.vector.tensor_add(dst[:rw], psp, bB[:rw])
            # two disjoint column groups in one bank: safe on HW (per-element
            # has_written), only the sim's zero-region check would object
            nc.tensor.matmul(gram_ps[:, :D], k1t[:rw], vt[:rw],
                             start=(r == 0), stop=(r == NR - 1),
                             skip_group_check=True)
            nc.tensor.matmul(gram_ps[:, D:2 * D], k2t[:rw], vt[:rw],
                             start=(r == 0), stop=(r == NR - 1),
                             skip_group_check=True)

        gram_sb = wtile([128, 2 * D], f32, "gram_sb")
        nc.vector.tensor_copy(gram_sb[:], gram_ps[:, :2 * D])

        # ---------------- gram all-reduce ----------------
        gr_in = dram.tile([128, 2 * D], f32, tag="gr_in", name="gr_in")
        gr_out = dram.tile([128, 2 * D], f32, tag="gr_out", name="gr_out",
                           addr_space=shared_space)
        nc.gpsimd.dma_start(out=gr_in[:], in_=gram_sb[:])
        nc.gpsimd.collective_compute("AllReduce", ALU.add, replica_groups=rg,
                                     ins=[gr_in.opt()], outs=[gr_out.opt()])

        def emit_watt():
            # gram AR completed long ago; safe to consume without stalling
            kv = wtile([128, 2 * D], f32, "kv")
            nc.scalar.dma_start(out=kv[:], in_=gr_out[:])
            psq1T = mm_tile()[:, :D]
            T(psq1T, Wq1[:], ident[:])
            Wq1T = wtile([128, D], f32, "Wq1T")
            nc.vector.tensor_copy(Wq1T[:], psq1T)
            psq2T = mm_tile()[:, :D]
            T(psq2T, Wq2[:], ident[:])
            Wq2T = wtile([128, D], f32, "Wq2T")
            nc.vector.tensor_copy(Wq2T[:], psq2T)

            ps_w1e = mm_tile()[:, :D]
            nc.tensor.matmul(ps_w1e, Wq1T[:], kv[:, :D])
            Watt = wtile([128, D], f32, "Watt")
            nc.vector.tensor_copy(Watt[:], ps_w1e)
            ps_w2e = mm_tile()[:, :D]
            nc.tensor.matmul(ps_w2e, Wq2T[:], kv[:, D:])
            tmp2 = wtile([128, D], f32, "tmp2")
            nc.vector.tensor_scalar(tmp2[:], ps_w2e, scalar1=neglam_c,
                                    scalar2=None, op0=ALU.mult)
            nc.vector.tensor_add(Watt[:], Watt[:], tmp2[:])

            ps_b1 = mm_tile()[:1, :D]
            nc.tensor.matmul(ps_b1, bq1_c[:], kv[:, :D])
            batt_r = wtile([1, D], f32, "batt_r")
            nc.vector.tensor_copy(batt_r[:], ps_b1)
            ps_b2 = mm_tile()[:1, :D]
            nc.tensor.matmul(ps_b2, bq2_c[:], kv[:, D:])
            tmpb = wtile([1, D], f32, "tmpb")
            nc.vector.tensor_scalar(tmpb[:], ps_b2, scalar1=neglam_c[:1],
                                    scalar2=None, op0=ALU.mult)
            nc.vector.tensor_add(batt_r[:], batt_r[:], tmpb[:])
            batt_B = bcast_row("batt_B", batt_r)
            return Watt, batt_B

        # ================= spectral pipeline =================
        p1_in, p1_out = [], []
        for c, (co, cw) in enumerate(chunks):
            p1_in.append(dram.tile([128, cw], bf16, tag=f"p1in{c}",
                                   name=f"p1in{c}"))
            p1_out.append(dram.tile([128, cw], bf16, tag=f"p1out{c}",
                                    name=f"p1out{c}", addr_space=shared_space))

        def emit_chunk_pass1(c):
            co, cw = chunks[c]
            cbs = _splits(cw, 512)
            ps1 = [p1_tile()[:, :bw] for bo, bw in cbs]
            for r, (ro, rw) in enumerate(ROWS):
                ut = ustream.tile([128, 2048], bf16, tag="u",
                                  name=f"u{c}_{r}")[:rw, :cw]
                nc.sync.dma_start(out=ut, in_=t_u16[ro:ro + rw, co:co + cw])
                for b, (bo, bw) in enumerate(cbs):
                    nc.tensor.matmul(ps1[b], h16[r][:rw], ut[:, bo:bo + bw],
                                     start=(r == 0), stop=(r == NR - 1))
            utxs = utxst.tile([128, 2048], bf16, tag="utxs",
                              name=f"utxs{c}")[:, :cw]
            for b, (bo, bw) in enumerate(cbs):
                nc.vector.tensor_copy(utxs[:, bo:bo + bw], ps1[b])
            nc.gpsimd.dma_start(out=p1_in[c][:], in_=utxs)
            nc.gpsimd.collective_compute(
                "AllReduce", ALU.add, replica_groups=rg,
                ins=[p1_in[c].opt()], outs=[p1_out[c].opt()])

        ps2_acc = []   # persistent accumulators for henc^T, one per i-block

        def emit_chunk_pass2(c):
            co, cw = chunks[c]
            if not ps2_acc:
                for io, iw in IBLK:
                    ps2_acc.append(p2_tile()[:, :iw])
            utxr = utxrd.tile([128, 2048], bf16, tag="utxr",
                              name=f"utxr{c}")[:, :cw]
            nc.scalar.dma_start(out=utxr, in_=p1_out[c][:])
            nsub_c = cw // 128
            for t in range(nsub_c):
                g = co // 128 + t
                # uT stream tile for this global subtile (no AR dependency)
                utt = utstream.tile([128, NLOC], bf16, tag="ut",
                                    name=f"ut{g}")
                nc.sync.dma_start(out=utt[:],
                                  in_=t_ut16[g * 128:(g + 1) * 128, :])
                # transpose the AR'd utx block to [k, d] and scale by ne
                pz = mmz_tile()[:, :D]
                T(pz, utxr[:, t * 128:(t + 1) * 128], identb[:])
                z = zpool.tile([128, D], bf16, tag="z", name=f"z{g}")
                nc.vector.tensor_scalar(z[:], pz, scalar1=ne[:, g:g + 1],
                                        scalar2=None, op0=ALU.mult)
                first = (g == 0)
                last = (g == NSUB - 1)
                for ib, (io, iw) in enumerate(IBLK):
                    nc.tensor.matmul(ps2_acc[ib], z[:], utt[:, io:io + iw],
                                     start=first, stop=last)

        # ========== chunk pipeline: pass2 DEPTH chunks behind pass1 ==========
        DEPTH = 2
        for c in range(NCH):
            emit_chunk_pass1(c)
            if c >= DEPTH:
                emit_chunk_pass2(c - DEPTH)

        # == attention (gram AR completed during early chunks) ==
        Watt, batt_B = emit_watt()
        ha = [wtile([128, D], f32, f"ha{r}") for r in range(NR)]
        s_sbs = []
        for r, (ro, rw) in enumerate(ROWS):
            pss = p1_tile()[:rw, :D]
            nc.tensor.matmul(pss, hnT[r][:, :rw], Watt[:])
            s_sb = wtile([128, D], f32, f"s_sb{r}")
            nc.vector.tensor_add(s_sb[:rw], pss, batt_B[:rw])
            layer_norm(s_sb[:rw], rw, s_sb[:rw])
            s_sbs.append(s_sb)
        for r, (ro, rw) in enumerate(ROWS):
            psT = mm_tile()[:, :rw]
            T(psT, s_sbs[r][:rw], ident[:rw, :rw])
            sT = rtile([128, 128], f32, "sT")
            nc.vector.tensor_copy(sT[:, :rw], psT)
            psa = p1_tile()[:rw, :D]
            nc.tensor.matmul(psa, sT[:, :rw], Wo[:])
            att = rtile([128, D], f32, "att")
            nc.vector.tensor_add(att[:rw], psa, bo_B[:rw])
            nc.vector.tensor_add(ha[r][:rw], h[r][:rw], att[:rw])

        for c in range(NCH - DEPTH, NCH):
            emit_chunk_pass2(c)

        # ================= residual + FFN =================
        hencT = wtile([128, NLOC], f32, "hencT")
        for ib, (io, iw) in enumerate(IBLK):
            nc.vector.tensor_copy(hencT[:, io:io + iw], ps2_acc[ib])

        mh = [wtile([128, D], f32, f"mh{r}") for r in range(NR)]
        gl = [wtile([128, D], f32, f"gl{r}") for r in range(NR)]
        for r, (ro, rw) in enumerate(ROWS):
            psb = mm_tile()[:rw, :D]
            T(psb, hencT[:, ro:ro + rw], ident[:])
            nc.vector.tensor_add(mh[r][:rw], ha[r][:rw], psb)
            fh = rtile([128, D], f32, "fh")
            layer_norm(mh[r][:rw], rw, fh[:rw])
            psT = p2_tile()[:, :rw]
            T(psT, fh[:rw], ident[:rw, :rw])
            fT = rtile([128, 128], f32, "fT")
            nc.vector.tensor_copy(fT[:, :rw], psT)
            psg = p1_tile()[:rw, :D]
            nc.tensor.matmul(psg, fT[:, :rw], W1p[:])
            nc.vector.tensor_add(gl[r][:rw], psg, b1p_B[:rw])
        for r, (ro, rw) in enumerate(ROWS):
            if sim_gelu:
                # tanh-approx gelu (CoreSim lacks Gelu); HW build uses AF.Gelu
                x3 = rtile([128, D], f32, "x3")
                nc.vector.tensor_mul(x3[:rw], gl[r][:rw], gl[r][:rw])
                nc.vector.tensor_mul(x3[:rw], x3[:rw], gl[r][:rw])
                nc.vector.tensor_scalar(x3[:rw], x3[:rw], scalar1=0.044715,
                                        scalar2=None, op0=ALU.mult)
                nc.vector.tensor_add(x3[:rw], x3[:rw], gl[r][:rw])
                nc.scalar.activation(x3[:rw], x3[:rw], AF.Tanh,
                                     scale=math.sqrt(2.0 / math.pi))
                nc.vector.tensor_scalar(x3[:rw], x3[:rw], scalar1=1.0,
                                        scalar2=0.5, op0=ALU.add, op1=ALU.mult)
                nc.vector.tensor_mul(gl[r][:rw], gl[r][:rw], x3[:rw])
            else:
                nc.scalar.activation(gl[r][:rw], gl[r][:rw], AF.Gelu)
        for r, (ro, rw) in enumerate(ROWS):
            psT2 = p2_tile()[:, :rw]
            T(psT2, gl[r][:rw], ident[:rw, :rw])
            gT = rtile([128, 128], f32, "gT")
            nc.vector.tensor_copy(gT[:, :rw], psT2)
            pso2 = p1_tile()[:rw, :D]
            nc.tensor.matmul(pso2, gT[:, :rw], f2w[:])
            outp = rtile([128, D], f32, "outp")
            nc.vector.tensor_add(outp[:rw], pso2, mh[r][:rw])
            nc.vector.tensor_add(outp[:rw], outp[:rw], f2b_B[:rw])
            nc.gpsimd.dma_start(out=t_out[ro:ro + rw, :], in_=outp[:rw])

    nc.compile()
    return nc


# ==================== host-side entry point ====================

_CACHED = {}


def _get_nc(N=N_FULL, NF=NF_FULL, CORES=CORES_FULL):
    key = (N, NF, CORES)
    if key not in _CACHED:
        _CACHED[key] = build_kernel(N, NF, CORES)
    return _CACHED[key]


def make_in_maps(inputs, N, CORES):
    import ml_dtypes

    bf16 = ml_dtypes.bfloat16
    NLOC = N // CORES
    full = {}
    for k, v in inputs.items():
        if k in ("u", "x"):
            continue
        full[k] = np.ascontiguousarray(np.asarray(v, dtype=np.float32))
    u = np.asarray(inputs["u"], dtype=np.float32)
    x = np.asarray(inputs["x"], dtype=np.float32)
    in_maps = []
    for c in range(CORES):
        rows = slice(c * NLOC, (c + 1) * NLOC)
        u_c = u[rows]
        u16 = np.zeros((NLOC, N_PAD), dtype=bf16)
        u16[:, :N] = u_c.astype(bf16)
        ut16 = np.zeros((N_PAD, NLOC), dtype=bf16)
        ut16[:N, :] = u_c.T.astype(bf16)
        xT = np.ascontiguousarray(x[rows].T)
        m = dict(full)
        m["u16"] = u16
        m["ut16"] = ut16
        m["xT"] = xT
        in_maps.append(m)
    return in_maps


def kernel(**inputs):
    from concourse import bass_utils

    nc = _get_nc()
    in_maps = make_in_maps(inputs, N_FULL, CORES_FULL)
    res = bass_utils.run_bass_kernel_spmd(nc, in_maps,
                                          core_ids=list(range(CORES_FULL)))
    out = np.concatenate([res.results[c]["out"] for c in range(CORES_FULL)],
                         axis=0)
    return out.astype(np.float32)


if __name__ == "__main__":
    build_kernel()
    print("build ok")
